# revision 1
# baseline (speedup 1.0000x reference)
"""Fused masked-attention kernel for Trainium2, data-parallel over batch on 8 cores.

Per core (one batch element): computes
  Q = query @ WQ.T ; K = key @ WK.T ; V = value @ WV.T      (H=64)
  S^T[k,q] = (K Q^T)[k,q]  (scores transposed, k on partitions)
  P^T = exp(S^T * 0.125) * notmask^T
  O_ext^T[h,q] = sum_k V_ext[k,h] P^T[k,q]   (V_ext has a ones column -> row 64 = Z)
  out[q,h] = O^T[h,q] / Z[q]   (via PE transpose + per-partition scalar mult)

Inputs are host-transposed (qT/kT/vT [E,L], inverted mask notmT [Lk,Lq]) so all
on-chip matmuls have their contraction dim on partitions with zero on-chip
transposes of large tensors. f32/u8 inputs are cast to fp16 during the (SWDGE)
DMA so every matmul runs at full PE rate; accumulation stays f32 in PSUM and
the softmax normalization stays f32. Q^T/K^T are zero-padded to K=128 so the
score matmuls drive all 128 PE rows (keeps the HAM activity monitor warm).
exp/mask-mult run 1024 wide (two 512-wide score tiles in adjacent PSUM banks)
to amortize per-instruction overhead on ACT/DVE.
"""

import numpy as np

import concourse.bass as bass
import concourse.tile as tile
from concourse import bacc, mybir
from concourse import bass_utils

B, L, E, H = 8, 4096, 1024, 64
NCORES = 8
F32 = mybir.dt.float32
F16 = mybir.dt.float16
U8 = mybir.dt.uint8

LB = 512  # l-block (free dim) for projections and q-blocks


def build_nc():
    nc = bacc.Bacc(
        "TRN2",
        target_bir_lowering=False,
        debug=False,
        enable_asserts=False,
        num_devices=NCORES,
    )
    qT = nc.dram_tensor("qT", [E, L], F16, kind="ExternalInput").ap()
    kT = nc.dram_tensor("kT", [E, L], F16, kind="ExternalInput").ap()
    vT = nc.dram_tensor("vT", [E, L], F16, kind="ExternalInput").ap()
    notmT = nc.dram_tensor("notmT", [L, L], U8, kind="ExternalInput").ap()
    wqT = nc.dram_tensor("wqT", [E, H], F16, kind="ExternalInput").ap()
    wkT = nc.dram_tensor("wkT", [E, H], F16, kind="ExternalInput").ap()
    wvT = nc.dram_tensor("wvT", [E, H], F16, kind="ExternalInput").ap()
    ident = nc.dram_tensor("ident", [128, 128], F32, kind="ExternalInput").ap()
    out = nc.dram_tensor("out", [L, H], F32, kind="ExternalOutput").ap()

    EXP = mybir.ActivationFunctionType.Exp
    NQB = L // LB

    qT_r = qT.rearrange("(c p) l -> p c l", p=128)
    kT_r = kT.rearrange("(c p) l -> p c l", p=128)
    vT_r = vT.rearrange("(c p) l -> p c l", p=128)
    notmT_r = notmT.rearrange("(c p) q -> p c q", p=128)

    with tile.TileContext(nc) as tc:
        with (
            tc.tile_pool(name="const", bufs=1) as constp,
            tc.tile_pool(name="persist", bufs=1) as persist,
            tc.tile_pool(name="kin", bufs=4) as kinp,
            tc.tile_pool(name="vin", bufs=4) as vinp,
            tc.tile_pool(name="qin", bufs=3) as qinp,
            tc.tile_pool(name="mask", bufs=2) as mpool,
            tc.tile_pool(name="pt", bufs=6) as ptpool,
            tc.tile_pool(name="osb", bufs=2) as opool,
            tc.tile_pool(name="zinv", bufs=4) as zpool,
            tc.tile_pool(name="otile", bufs=4) as otpool,
            tc.tile_pool(name="ps_st", bufs=2, space="PSUM") as ps_st,
            tc.tile_pool(name="ps_o", bufs=1, space="PSUM") as ps_o,
            tc.tile_pool(name="ps_small", bufs=3, space="PSUM") as ps_small,
        ):
            ident_sb = constp.tile([128, 128], F32)
            nc.sync.dma_start(ident_sb[:], ident)
            # weights, e-chunked: [128, 8, 64], cast to fp16 during DMA
            wq_sb = constp.tile([128, 8, H], F16)
            wk_sb = constp.tile([128, 8, H], F16)
            wv_sb = constp.tile([128, 8, H], F16)
            nc.sync.dma_start(wq_sb[:], wqT.rearrange("(c p) h -> p c h", p=128))
            nc.sync.dma_start(wk_sb[:], wkT.rearrange("(c p) h -> p c h", p=128))
            nc.sync.dma_start(wv_sb[:], wvT.rearrange("(c p) h -> p c h", p=128))

            # Q^T/K^T [h, l] zero-padded to 128 rows (full-row score matmuls)
            QT_sb = persist.tile([128, L], F16)
            KT_sb = persist.tile([128, L], F16)
            nc.vector.memset(QT_sb[64:128, :], 0.0)
            nc.vector.memset(KT_sb[64:128, :], 0.0)
            # V [k, h] + ones col, padded to 128 cols so AV weight loads
            # use FWL (needs exactly 128 columns); pad cols are zeroed and
            # the corresponding PSUM rows 65-127 are never read.
            V_sb = persist.tile([128, 32, 128], F16)
            nc.vector.memset(V_sb[:, :, H : 128], 0.0)
            nc.vector.memset(V_sb[:, :, H : H + 1], 1.0)

            # ---------------- Phase 1: K/V loads + projections ----------------
            # All of K first so the first score matmuls can start while V is
            # still loading/projecting.
            for lb in range(L // LB):
                ls = lb * LB
                k_in = kinp.tile([128, 8, LB], F16, tag="kin")
                nc.sync.dma_start(k_in[:], kT_r[:, :, ls : ls + LB])
                p_kt = ps_small.tile([64, LB], F32, tag="small")
                for ec in range(8):
                    nc.tensor.matmul(
                        p_kt[:], wk_sb[:, ec, :], k_in[:, ec, :],
                        start=(ec == 0), stop=(ec == 7),
                    )
                nc.scalar.copy(KT_sb[0:64, ls : ls + LB], p_kt[:])
            for lb in range(L // LB):
                ls = lb * LB
                v_in = vinp.tile([128, 8, LB], F16, tag="vin")
                nc.sync.dma_start(v_in[:], vT_r[:, :, ls : ls + LB])
                # V: [k,h] layout -> stationary = vT chunk, moving = wvT chunk
                for sub in range(LB // 128):
                    p_v = ps_small.tile([128, H], F32, tag="small")
                    for ec in range(8):
                        nc.tensor.matmul(
                            p_v[:],
                            v_in[:, ec, sub * 128 : (sub + 1) * 128],
                            wv_sb[:, ec, :],
                            start=(ec == 0),
                            stop=(ec == 7),
                        )
                    nc.scalar.copy(V_sb[:, lb * 4 + sub, 0:H], p_v[:])

            # ---------------- Phase 2: Q proj + scores/softmax/AV ----------------
            def load_q(qb):
                qs = qb * LB
                q_in = qinp.tile([128, 8, LB], F16, tag="qin")
                nc.sync.dma_start(q_in[:], qT_r[:, :, qs : qs + LB])
                return q_in

            def proj_q(qb, q_in):
                qs = qb * LB
                p_qt = ps_small.tile([64, LB], F32, tag="small")
                for ec in range(8):
                    nc.tensor.matmul(
                        p_qt[:], wq_sb[:, ec, :], q_in[:, ec, :],
                        start=(ec == 0), stop=(ec == 7),
                    )
                nc.scalar.copy(QT_sb[0:64, qs : qs + LB], p_qt[:])

            def load_mask(qb):
                qs = qb * LB
                mtile = mpool.tile([128, 32 * LB], F16, tag="m")
                m3 = mtile[:].rearrange("p (c q) -> p c q", q=LB)
                for quarter in range(4):
                    nc.gpsimd.dma_start(
                        m3[:, 8 * quarter : 8 * (quarter + 1), :],
                        notmT_r[:, 8 * quarter : 8 * (quarter + 1), qs : qs + LB],
                    )
                return mtile

            def epilogue(qb, p_o):
                qs = qb * LB
                o_sb = opool.tile([H + 1, LB], F32, tag="o_sb")
                nc.scalar.copy(o_sb[:], p_o[0 : H + 1, :])
                for sub in range(LB // 128):
                    p_t = ps_small.tile([128, H + 1], F32, tag="small")
                    nc.tensor.transpose(
                        p_t[:],
                        o_sb[:, sub * 128 : (sub + 1) * 128],
                        ident_sb[0 : H + 1, 0 : H + 1],
                    )
                    zinv = zpool.tile([128, 1], F32, tag="zinv")
                    nc.vector.reciprocal(zinv[:], p_t[:, H : H + 1])
                    ot = otpool.tile([128, H], F32, tag="ot")
                    nc.vector.tensor_scalar_mul(ot[:], p_t[:, 0:H], zinv[:])
                    r0 = qs + sub * 128
                    nc.sync.dma_start(out[r0 : r0 + 128, :], ot[:])

            # prologue: q/mask for block 0, q for block 1 (overlaps phase 1)
            q0 = load_q(0)
            m0 = load_mask(0)
            proj_q(0, q0)
            q_next = load_q(1)

            # dense PE warmup burst to trip the HAM SHORT busy window
            p_w = ps_st.tile([128, 128], F32, tag="p_st")
            for w in range(32):
                nc.tensor.matmul(
                    p_w[:], V_sb[:, 31, :], KT_sb[:, 0:128],
                    start=True, stop=True,
                )

            pending = None  # (qb, p_o) awaiting epilogue
            mtile = m0
            for qb in range(NQB):
                qs = qb * LB
                p_o = ps_o.tile([128, LB], F32, tag="p_o")
                m_next = None
                for g in range(8):  # groups of 4 k-chunks (2 wide pairs)
                    wides = []
                    for j in range(2):
                        kc = 4 * g + 2 * j
                        p_st = ps_st.tile([128, 2 * LB], F32, tag="p_st")
                        nc.tensor.matmul(
                            p_st[:, 0:LB],
                            KT_sb[:, kc * 128 : (kc + 1) * 128],
                            QT_sb[:, qs : qs + LB],
                            start=True,
                            stop=True,
                        )
                        nc.tensor.matmul(
                            p_st[:, LB : 2 * LB],
                            KT_sb[:, (kc + 1) * 128 : (kc + 2) * 128],
                            QT_sb[:, qs : qs + LB],
                            start=True,
                            stop=True,
                        )
                        wides.append((kc, p_st))
                    if g == 1 and pending is not None:
                        epilogue(*pending)
                        pending = None
                    if g == 2 and qb + 1 < NQB:
                        m_next = load_mask(qb + 1)
                    if g == 4 and qb + 1 < NQB:
                        proj_q(qb + 1, q_next)
                    if g == 6 and qb + 2 < NQB:
                        q_next = load_q(qb + 2)
                    pts = []
                    for kc, p_st in wides:
                        pt = ptpool.tile([128, 2 * LB], F16, tag="pt")
                        nc.scalar.activation(pt[:], p_st[:], EXP, scale=0.125)
                        nc.vector.tensor_mul(
                            pt[:], pt[:], mtile[:, kc * LB : (kc + 2) * LB]
                        )
                        pts.append((kc, pt))
                    for kc, pt in pts:
                        nc.tensor.matmul(
                            p_o[:], V_sb[:, kc, :], pt[:, 0:LB],
                            start=(kc == 0), stop=False,
                        )
                        nc.tensor.matmul(
                            p_o[:], V_sb[:, kc + 1, :], pt[:, LB : 2 * LB],
                            start=False, stop=(kc + 1 == 31),
                        )
                pending = (qb, p_o)
                mtile = m_next
            epilogue(*pending)
    nc.compile()
    return nc


_NC_CACHE = {}


def kernel(query, key, value, mask, WQ, WK, WV):
    if "nc" not in _NC_CACHE:
        _NC_CACHE["nc"] = build_nc()
    nc = _NC_CACHE["nc"]

    ident = np.eye(128, dtype=np.float32)
    wqT = np.ascontiguousarray(np.asarray(WQ, dtype=np.float16).T)
    wkT = np.ascontiguousarray(np.asarray(WK, dtype=np.float16).T)
    wvT = np.ascontiguousarray(np.asarray(WV, dtype=np.float16).T)
    notm = ~np.asarray(mask)  # True where attention is allowed
    in_maps = []
    for b in range(B):
        in_maps.append(
            {
                "qT": np.ascontiguousarray(np.asarray(query[b], dtype=np.float16).T),
                "kT": np.ascontiguousarray(np.asarray(key[b], dtype=np.float16).T),
                "vT": np.ascontiguousarray(np.asarray(value[b], dtype=np.float16).T),
                "notmT": np.ascontiguousarray(notm[b].T).view(np.uint8),
                "wqT": wqT,
                "wkT": wkT,
                "wvT": wvT,
                "ident": ident,
            }
        )
    res = bass_utils.run_bass_kernel_spmd(nc, in_maps, core_ids=list(range(NCORES)))
    out = np.stack([res.results[b]["out"] for b in range(B)], axis=0)
    return out


if __name__ == "__main__":
    rng = np.random.default_rng(0)
    q = rng.standard_normal((B, L, E), dtype=np.float32)
    k = rng.standard_normal((B, L, E), dtype=np.float32)
    v = rng.standard_normal((B, L, E), dtype=np.float32)
    m = rng.integers(0, 2, size=(B, L, L)).astype(bool)
    s = 1.0 / np.sqrt(E)
    wq = rng.uniform(-s, s, size=(H, E)).astype(np.float32)
    wk = rng.uniform(-s, s, size=(H, E)).astype(np.float32)
    wv = rng.uniform(-s, s, size=(H, E)).astype(np.float32)
    o = kernel(query=q, key=k, value=v, mask=m, WQ=wq, WK=wk, WV=wv)
    print(o.shape, o.dtype)



# revision 10
# speedup vs baseline: 1.0228x; 1.0228x over previous
"""Fused masked-attention kernel for Trainium2, data-parallel over batch on 8 cores.

v3 design notes (all per core; one batch element per core):
- DMA dest-side bytes are the primary constraint (16 DMA engines). All host
  layouts are arranged so every DMA descriptor is a >=8KB contiguous
  per-partition run (the baseline's 1KB packets capped fleet rate).
- The bool mask ships as u8 {0,1} (FORBIDDEN positions) and is used directly
  as the predicate of copy_predicated, which zeroes masked entries of exp(S)
  in place on DVE. No cast DMA (u8->f16 doubled dest bytes in the baseline),
  no unpack ops.
- Scores: row-tiled matmul pairs. QT/KT live duplicated in both partition
  halves (via [w|w]-duplicated projection weights), so chunk pairs run as two
  concurrent K=64 matmuls on disjoint PE row-groups (HW row-group tiling).
- AV accumulates f32 in PSUM over 32 k-chunks per 512-wide q block; V carries
  a ones column so PSUM row 64 is Z. Output ships unnormalized O^T+Z [65, L]
  f32; host does the divide + transpose.
- exp on ACT; copy_predicated on DVE; projections evacuate via DVE.
"""

import numpy as np

import concourse.bass as bass
import concourse.tile as tile
from concourse import bacc, mybir
from concourse import bass_utils

B, L, E, H = 8, 4096, 1024, 64
NCORES = 8
F32 = mybir.dt.float32
F16 = mybir.dt.float16
U8 = mybir.dt.uint8

LB = 512           # q-block and projection block width
NQB = L // LB      # 8
NCH = L // 128     # 32 k-chunks
NEC = E // 128     # 8 e-chunks


def build_nc():
    nc = bacc.Bacc(
        "TRN2",
        target_bir_lowering=False,
        debug=False,
        enable_asserts=False,
        num_devices=NCORES,
    )
    # host pre-shuffled layouts: per-partition runs are fully contiguous
    q2 = nc.dram_tensor("q2", [NQB, 128, NEC, LB], F16, kind="ExternalInput").ap()
    k2 = nc.dram_tensor("k2", [NQB, 128, NEC, LB], F16, kind="ExternalInput").ap()
    v2 = nc.dram_tensor("v2", [NQB, 128, NEC, LB], F16, kind="ExternalInput").ap()
    mu8 = nc.dram_tensor("mu8", [NQB, 128, NCH, LB], U8, kind="ExternalInput").ap()
    wqD = nc.dram_tensor("wqD", [E, 128], F16, kind="ExternalInput").ap()
    wkD = nc.dram_tensor("wkD", [E, 128], F16, kind="ExternalInput").ap()
    wvT = nc.dram_tensor("wvT", [E, H], F16, kind="ExternalInput").ap()
    out = nc.dram_tensor("out", [H + 1, L], F32, kind="ExternalOutput").ap()

    EXP = mybir.ActivationFunctionType.Exp

    with tile.TileContext(nc) as tc:
        with (
            tc.tile_pool(name="const", bufs=1) as constp,
            tc.tile_pool(name="persist", bufs=1) as persist,
            tc.tile_pool(name="kin", bufs=2) as kinp,
            tc.tile_pool(name="vin", bufs=2) as vinp,
            tc.tile_pool(name="qin", bufs=2) as qinp,
            tc.tile_pool(name="mpk", bufs=2) as mpool,
            tc.tile_pool(name="pt", bufs=8) as ptpool,
            tc.tile_pool(name="osb", bufs=2) as opool,
            tc.tile_pool(name="ps_st", bufs=2, space="PSUM") as ps_st,
            tc.tile_pool(name="ps_o", bufs=3, space="PSUM") as ps_o,
            tc.tile_pool(name="ps_pj", bufs=1, space="PSUM") as ps_pj,
        ):
            # ---- constants / weights ----
            wq_sb = constp.tile([128, NEC, 128], F16)
            wk_sb = constp.tile([128, NEC, 128], F16)
            wv_sb = constp.tile([128, NEC, H], F16)
            nc.sync.dma_start(wq_sb[:], wqD.rearrange("(c p) h -> p c h", p=128))
            nc.sync.dma_start(wk_sb[:], wkD.rearrange("(c p) h -> p c h", p=128))
            nc.sync.dma_start(wv_sb[:], wvT.rearrange("(c p) h -> p c h", p=128))

            # persistent projected tensors
            QT_sb = persist.tile([128, L], F16)   # rows 0:64 = Q^T, 64:128 = copy
            KT_sb = persist.tile([128, L], F16)
            V_sb = persist.tile([128, NCH, 128], F16)  # [k, h] + ones col 64, pad 0
            nc.vector.memset(V_sb[:, :, H : 128], 0.0)
            nc.vector.memset(V_sb[:, :, H : H + 1], 1.0)
            zero_sb = constp.tile([128, 1], F16)
            nc.vector.memset(zero_sb[:], 0.0)

            # mask for qb0
            mpk_sb0 = mpool.tile([128, NCH, LB], U8, tag="mpk")
            nc.sync.dma_start(mpk_sb0[:], mu8[0])

            # ---- PE warmup on weights (HAM) ----
            p_w = ps_st.tile([128, 1024], F32, tag="p_st")
            for w in range(64):
                nc.tensor.matmul(
                    p_w[:, 0:128], wq_sb[:, 0, :], wq_sb[:, 0, 0:128],
                    start=True, stop=True,
                )

            def proj_k_block(b, k_in):
                ls = b * LB
                p_pj = ps_pj.tile([128, LB], F32, tag="pj")
                for ec in range(NEC):
                    nc.tensor.matmul(
                        p_pj[:], wk_sb[:, ec, :], k_in[:, ec, :],
                        start=(ec == 0), stop=(ec == NEC - 1),
                    )
                nc.vector.tensor_copy(KT_sb[:, ls : ls + LB], p_pj[:])

            def proj_q_block(b, q_in):
                ls = b * LB
                p_pj = ps_pj.tile([128, LB], F32, tag="pj")
                for ec in range(NEC):
                    nc.tensor.matmul(
                        p_pj[:], wq_sb[:, ec, :], q_in[:, ec, :],
                        start=(ec == 0), stop=(ec == NEC - 1),
                    )
                nc.vector.tensor_copy(QT_sb[:, ls : ls + LB], p_pj[:])

            def proj_v_block(b, v_in):
                for sub in range(4):
                    p_pj = ps_pj.tile([128, LB], F32, tag="pj")
                    for ec in range(NEC):
                        nc.tensor.matmul(
                            p_pj[:, 0:H],
                            v_in[:, ec, sub * 128 : (sub + 1) * 128],
                            wv_sb[:, ec, :],
                            start=(ec == 0), stop=(ec == NEC - 1),
                        )
                    nc.vector.tensor_copy(V_sb[:, b * 4 + sub, 0:H], p_pj[:, 0:H])

            # ---- startup: stream K/V/q0 and project ----
            def load_kv(b):
                k_in = kinp.tile([128, NEC, LB], F16, tag="kin")
                nc.sync.dma_start(k_in[:], k2[b])
                v_in = vinp.tile([128, NEC, LB], F16, tag="vin")
                nc.sync.dma_start(v_in[:], v2[b])
                return k_in, v_in

            def load_q(b):
                q_in = qinp.tile([128, NEC, LB], F16, tag="qin")
                nc.sync.dma_start(q_in[:], q2[b])
                return q_in

            q0 = load_q(0)
            for b in range(NQB):
                k_in, v_in = load_kv(b)
                proj_k_block(b, k_in)
                proj_v_block(b, v_in)
            proj_q_block(0, q0)

            # ---- main loop ----
            mtile = mpk_sb0
            q_next = None
            for qb in range(NQB):
                qs = qb * LB
                p_o = ps_o.tile([128, LB], F32, tag="p_o")
                m_next = None
                for g in range(16):
                    cA, cB = 2 * g, 2 * g + 1
                    ps = ps_st.tile([128, 1024], F32, tag="p_st")
                    # two concurrent K=64 row-tiled score matmuls (N=512)
                    nc.tensor.matmul(
                        ps[:, 0:512],
                        KT_sb[0:64, cA * 128 : (cA + 1) * 128],
                        QT_sb[0:64, qs : qs + LB],
                        start=True, stop=True,
                    )
                    nc.tensor.matmul(
                        ps[:, 512:1024],
                        KT_sb[64:128, cB * 128 : (cB + 1) * 128],
                        QT_sb[64:128, qs : qs + LB],
                        start=True, stop=True,
                    )
                    # exp on ACT
                    pt = ptpool.tile([128, 1024], F16, tag="pt")
                    nc.scalar.activation(pt[:], ps[:], EXP, scale=0.125)
                    # zero forbidden entries in place (u8 mask as predicate)
                    nc.vector.copy_predicated(
                        pt[:],
                        mtile[:, cA : cA + 2, :].rearrange("p c q -> p (c q)"),
                        zero_sb[:].to_broadcast([128, 1024]),
                    )
                    # prefetch hooks
                    if g == 1 and qb + 1 < NQB:
                        m_next = mpool.tile([128, NCH, LB], U8, tag="mpk")
                        nc.sync.dma_start(m_next[:], mu8[qb + 1])
                    if g == 4 and qb + 1 < NQB:
                        q_next = load_q(qb + 1)
                    if g == 8 and qb + 1 < NQB:
                        proj_q_block(qb + 1, q_next)
                    # AV: accumulate both chunks
                    nc.tensor.matmul(
                        p_o[:], V_sb[:, cA, :], pt[:, 0:512],
                        start=(g == 0), stop=False,
                    )
                    nc.tensor.matmul(
                        p_o[:], V_sb[:, cB, :], pt[:, 512:1024],
                        start=False, stop=(g == 15),
                    )
                # epilogue: ship unnormalized O^T + Z row
                o_sb = opool.tile([H + 1, LB], F32, tag="osb")
                nc.scalar.copy(o_sb[:], p_o[0 : H + 1, :])
                nc.sync.dma_start(out[:, qs : qs + LB], o_sb[:])
                mtile = m_next
    nc.compile()
    return nc


_NC_CACHE = {}


def _shuffle_pcl(xT):
    """xT: [E, L] -> [NQB, 128, NEC, LB], so partition p of block b holds
    e-rows {c*128+p} as contiguous 512-col runs."""
    a = xT.reshape(NEC, 128, NQB, LB)      # [c, p, b, l']
    return np.ascontiguousarray(a.transpose(2, 1, 0, 3))


def _shuffle_mask(forb_b):
    """forb_b: [L, L] bool (True = forbidden), indexed [q, k].
    Returns [NQB, 128, NCH, LB] u8: [qb, p, c, q'] = forb[qb*512+q', c*128+p]."""
    A = forb_b.T.reshape(NCH, 128, NQB, LB)  # [c, p, qb, q']
    return np.ascontiguousarray(A.transpose(2, 1, 0, 3)).astype(np.uint8)


def kernel(query, key, value, mask, WQ, WK, WV):
    if "nc" not in _NC_CACHE:
        _NC_CACHE["nc"] = build_nc()
    nc = _NC_CACHE["nc"]

    wqT = np.asarray(WQ, dtype=np.float16).T  # [E, H]
    wkT = np.asarray(WK, dtype=np.float16).T
    wvT = np.ascontiguousarray(np.asarray(WV, dtype=np.float16).T)
    wqD = np.ascontiguousarray(np.concatenate([wqT, wqT], axis=1))
    wkD = np.ascontiguousarray(np.concatenate([wkT, wkT], axis=1))
    forb = np.asarray(mask)  # [B, L, L], True where forbidden
    in_maps = []
    for b in range(B):
        in_maps.append(
            {
                "q2": _shuffle_pcl(np.asarray(query[b], dtype=np.float16).T),
                "k2": _shuffle_pcl(np.asarray(key[b], dtype=np.float16).T),
                "v2": _shuffle_pcl(np.asarray(value[b], dtype=np.float16).T),
                "mu8": _shuffle_mask(forb[b]),
                "wqD": wqD,
                "wkD": wkD,
                "wvT": wvT,
            }
        )
    res = bass_utils.run_bass_kernel_spmd(nc, in_maps, core_ids=list(range(NCORES)))
    outs = []
    for b in range(B):
        ot = res.results[b]["out"].astype(np.float64)  # [65, L]
        o = (ot[0:H] / ot[H : H + 1]).T  # [L, H]
        outs.append(o.astype(np.float32))
    return np.stack(outs, axis=0)


if __name__ == "__main__":
    rng = np.random.default_rng(0)
    q = rng.standard_normal((B, L, E), dtype=np.float32)
    k = rng.standard_normal((B, L, E), dtype=np.float32)
    v = rng.standard_normal((B, L, E), dtype=np.float32)
    m = rng.integers(0, 2, size=(B, L, L)).astype(bool)
    s = 1.0 / np.sqrt(E)
    wq = rng.uniform(-s, s, size=(H, E)).astype(np.float32)
    wk = rng.uniform(-s, s, size=(H, E)).astype(np.float32)
    wv = rng.uniform(-s, s, size=(H, E)).astype(np.float32)
    o = kernel(query=q, key=k, value=v, mask=m, WQ=wq, WK=wk, WV=wv)
    print(o.shape, o.dtype)
